# revision 22
# baseline (speedup 1.0000x reference)
"""GRU free-run greedy decoder on 8 Trainium2 NeuronCores (data parallel).

Problem: 2-layer GRU (H=512) + fc(V=256) greedy decode, T=64 steps,
B=1024 batch, latent LAT=256 concatenated with previous one-hot as input.

Sharding: pure data parallel. Each of the 8 cores handles 128 batch rows
(= exactly the 128 SBUF partitions). GRU + fc weights are replicated.
The whole recurrence runs on-chip: weights, hidden state, and per-step
one-hots all live in SBUF; only the final [128, T, V] one-hot stream is
DMA'd out.

Matmul mapping ("mapping 1"): out[batch, outdim] = lhsT.T @ rhs with
  lhsT (stationary) = activation^T chunk [K=128, 128 batch]
  rhs  (moving)     = weight^T chunk     [K=128, <=512 outdim]
so the PE streams the (large) weight operand and the per-step activation
transposes are small PE transpose ops. h-dependent matmuls are fp32 (the
argmax trajectory needs fp32-faithful logits; bf16/tf32/f32r flip tokens
— measured). One-hot embedding and bias adds run as EXACT 3-way bf16
decompositions at full PE rate, seeded into PSUM by prefetchable
identity-matmuls.

Host-side prep (layout + exact splits): weight transposes / reshapes,
bias combination + replication across partitions.
"""

import sys
import numpy as np

sys.path.insert(0, "/opt/trn_rl_repo")

P = 128          # partitions == per-core batch
H = 512          # hidden
V = 256          # vocab
LAT = 256        # latent dim
G3 = 3 * H       # 1536 gate width
T_FULL = 64
N_CORES = 8

_CACHE = {}


def build_program(T=T_FULL, use_f32r=False):
    """Build + compile the Bass program. Returns the compiled Bacc object."""
    import concourse.bass as bass
    import concourse.tile as tile
    from concourse import bacc, mybir
    from concourse.masks import make_identity

    f32 = mybir.dt.float32
    bf16 = mybir.dt.bfloat16
    mm = mybir.dt.float32r if use_f32r else f32
    AF = mybir.ActivationFunctionType
    OP = mybir.AluOpType
    ts = bass.ts

    nc = bacc.Bacc(
        "TRN2", target_bir_lowering=False, debug=False,
        enable_asserts=False, num_devices=N_CORES,
    )

    # ---- DRAM I/O ----
    # one-hot/identity/bias matmuls run as EXACT 3-way bf16 decompositions
    # (8+8+8 mantissa bits cover fp32's 24; measured: even 2^-18 rounding
    # here flips argmax rows). h-dependent gate/fc matmuls stay fp32.
    lat_d = nc.dram_tensor("lat", [P, LAT], f32, kind="ExternalInput").ap()
    wlatT_d = nc.dram_tensor("wlatT", [2, P, G3], mm, kind="ExternalInput").ap()
    wembTh_d = nc.dram_tensor("wembTh", [2, P, G3], bf16, kind="ExternalInput").ap()
    wembTm_d = nc.dram_tensor("wembTm", [2, P, G3], bf16, kind="ExternalInput").ap()
    wembTl_d = nc.dram_tensor("wembTl", [2, P, G3], bf16, kind="ExternalInput").ap()
    whh0T_d = nc.dram_tensor("whh0T", [4, P, G3], mm, kind="ExternalInput").ap()
    wih1T_d = nc.dram_tensor("wih1T", [4, P, G3], mm, kind="ExternalInput").ap()
    whh1T_d = nc.dram_tensor("whh1T", [4, P, G3], mm, kind="ExternalInput").ap()
    wfcT_d = nc.dram_tensor("wfcT", [4, P, V], mm, kind="ExternalInput").ap()
    blc_d = nc.dram_tensor("blc", [P, G3], f32, kind="ExternalInput").ap()
    b0hn_d = nc.dram_tensor("b0hn", [P, H], f32, kind="ExternalInput").ap()
    b1rzh_d = nc.dram_tensor("b1rzh", [P, 2 * H], bf16, kind="ExternalInput").ap()
    b1rzm_d = nc.dram_tensor("b1rzm", [P, 2 * H], bf16, kind="ExternalInput").ap()
    b1rzl_d = nc.dram_tensor("b1rzl", [P, 2 * H], bf16, kind="ExternalInput").ap()
    b1in_d = nc.dram_tensor("b1in", [P, H], f32, kind="ExternalInput").ap()
    b1hn_d = nc.dram_tensor("b1hn", [P, H], f32, kind="ExternalInput").ap()
    bfch_d = nc.dram_tensor("bfch", [P, V], bf16, kind="ExternalInput").ap()
    bfcm_d = nc.dram_tensor("bfcm", [P, V], bf16, kind="ExternalInput").ap()
    bfcl_d = nc.dram_tensor("bfcl", [P, V], bf16, kind="ExternalInput").ap()
    out_d = nc.dram_tensor("out", [P, T, V], f32, kind="ExternalOutput").ap()

    from contextlib import ExitStack
    with tile.TileContext(nc) as tc, ExitStack() as ctx:
        wt = ctx.enter_context(tc.tile_pool(name="wt", bufs=1))
        st = ctx.enter_context(tc.tile_pool(name="st", bufs=1))
        wk = ctx.enter_context(tc.tile_pool(name="wk", bufs=2))
        # PSUM: 8 banks total. rz gates 2x[P,1024] double-buffered (4
        # banks) + ihn 2x[P,1024] (4 banks). Transposes/fc/ohT recycle the
        # ihn tiles' regions after their gate reads complete.
        ps = ctx.enter_context(tc.tile_pool(name="ps", bufs=2, space="PSUM"))

        # ---- persistent weights/biases in SBUF ----
        whh0T = wt.tile([P, 4, G3], mm, tag="whh0T")
        wih1T = wt.tile([P, 4, G3], mm, tag="wih1T")
        whh1T = wt.tile([P, 4, G3], mm, tag="whh1T")
        wembTh = wt.tile([P, 2, G3], bf16, tag="wembTh")
        wembTm = wt.tile([P, 2, G3], bf16, tag="wembTm")
        wembTl = wt.tile([P, 2, G3], bf16, tag="wembTl")
        wlatT = wt.tile([P, 2, G3], mm, tag="wlatT")
        wfcT = wt.tile([P, 4, V], mm, tag="wfcT")
        # DMA order matches first-use: step 0 needs the Lc chain + wih1T
        # + wfcT (hh0/gh1/emb are skipped at t=0), step 1 the rest.
        latsb = wt.tile([P, LAT], f32, tag="latsb")
        nc.sync.dma_start(latsb[:], lat_d[:])
        for kc in range(2):
            nc.sync.dma_start(wlatT[:, kc, :], wlatT_d[kc])
        for kc in range(4):
            nc.sync.dma_start(wih1T[:, kc, :], wih1T_d[kc])
        for kc in range(4):
            nc.sync.dma_start(wfcT[:, kc, :], wfcT_d[kc])
        for kc in range(4):
            nc.sync.dma_start(whh0T[:, kc, :], whh0T_d[kc])
            nc.sync.dma_start(whh1T[:, kc, :], whh1T_d[kc])
        for kc in range(2):
            nc.sync.dma_start(wembTh[:, kc, :], wembTh_d[kc])
            nc.sync.dma_start(wembTm[:, kc, :], wembTm_d[kc])
            nc.sync.dma_start(wembTl[:, kc, :], wembTl_d[kc])

        blc = wt.tile([P, G3], f32, tag="blc")
        b0hn = wt.tile([P, H], f32, tag="b0hn")
        b1rzh = wt.tile([P, 2 * H], bf16, tag="b1rzh")
        b1rzm = wt.tile([P, 2 * H], bf16, tag="b1rzm")
        b1rzl = wt.tile([P, 2 * H], bf16, tag="b1rzl")
        b1in = wt.tile([P, H], f32, tag="b1in")
        b1hn = wt.tile([P, H], f32, tag="b1hn")
        bfch = wt.tile([P, V], bf16, tag="bfch")
        bfcm = wt.tile([P, V], bf16, tag="bfcm")
        bfcl = wt.tile([P, V], bf16, tag="bfcl")
        nc.sync.dma_start(blc[:], blc_d[:])
        nc.sync.dma_start(b0hn[:], b0hn_d[:])
        nc.sync.dma_start(b1rzh[:], b1rzh_d[:])
        nc.sync.dma_start(b1rzm[:], b1rzm_d[:])
        nc.sync.dma_start(b1rzl[:], b1rzl_d[:])
        nc.sync.dma_start(b1in[:], b1in_d[:])
        nc.sync.dma_start(b1hn[:], b1hn_d[:])
        nc.sync.dma_start(bfch[:], bfch_d[:])
        nc.sync.dma_start(bfcm[:], bfcm_d[:])
        nc.sync.dma_start(bfcl[:], bfcl_d[:])

        zer = wt.tile([P, H], bf16, tag="zer")
        nc.gpsimd.memset(zer[:], 0.0)
        ident = wt.tile([P, P], f32, tag="ident")
        make_identity(nc, ident[:])
        identb = wt.tile([P, P], bf16, tag="identb")
        make_identity(nc, identb[:])

        # ---- persistent state ----
        h0 = st.tile([P, H], f32, tag="h0")
        h1 = st.tile([P, H], f32, tag="h1")
        h0T = st.tile([P, 4, P], mm, tag="h0T")
        h1T = st.tile([P, 4, P], mm, tag="h1T")
        ohT = st.tile([P, 2, P], bf16, tag="ohT")
        Lc = st.tile([P, G3], f32, tag="Lc")
        Lch = st.tile([P, 2 * H], bf16, tag="Lch")  # rz part, hi
        Lcm = st.tile([P, 2 * H], bf16, tag="Lcm")  # rz part, mid
        Lcl = st.tile([P, 2 * H], bf16, tag="Lcl")  # rz part, lo
        for tl in (h0, h1):
            nc.gpsimd.memset(tl[:], 0.0)
        nc.gpsimd.memset(h0T[:, :, :], 0.0)
        nc.gpsimd.memset(h1T[:, :, :], 0.0)
        nc.gpsimd.memset(ohT[:, :, :], 0.0)

        # ---- setup: Lc = latent @ WlatT + (b_ih0 + b_hh0 (rz-only)) ----
        s1 = ps.tile([P, 1024], f32, tag="rz")
        latT = wt.tile([P, 2, P], mm, tag="latT")
        for kc in range(2):
            nc.tensor.transpose(s1[:, ts(kc, P)], latsb[:, ts(kc, P)], ident[:])
        nc.scalar.copy(latT[:, :, :].rearrange("p a b -> p (a b)"), s1[:, 0:256])

        s2 = ps.tile([P, 1024], f32, tag="rz")
        s3 = ps.tile([P, 1024], f32, tag="ihn")
        for kc in range(2):
            for j in range(2):
                nc.tensor.matmul(s2[:, ts(j, 512)], latT[:, kc, :],
                                 wlatT[:, kc, ts(j, 512)],
                                 start=(kc == 0), stop=(kc == 1))
            nc.tensor.matmul(s3[:, 0:512], latT[:, kc, :],
                             wlatT[:, kc, 1024:1536],
                             start=(kc == 0), stop=(kc == 1))
        nc.vector.tensor_add(Lc[:, 0:1024], s2[:, 0:1024], blc[:, 0:1024])
        nc.vector.tensor_add(Lc[:, 1024:1536], s3[:, 0:512], blc[:, 1024:1536])
        # split the rz part into an EXACT 3-way bf16 sum (8+8+8 mantissa
        # bits cover fp32's 24) for the per-step psum-seed matmul
        nc.vector.tensor_copy(Lch[:], Lc[:, 0:1024])
        Lchf = wt.tile([P, 2 * H], f32, tag="Lchf")
        nc.vector.tensor_copy(Lchf[:], Lch[:])
        r1 = wt.tile([P, 2 * H], f32, tag="r1")
        nc.vector.tensor_sub(r1[:], Lc[:, 0:1024], Lchf[:])
        nc.vector.tensor_copy(Lcm[:], r1[:])
        Lcmf = wt.tile([P, 2 * H], f32, tag="Lcmf")
        nc.vector.tensor_copy(Lcmf[:], Lcm[:])
        nc.vector.tensor_sub(r1[:], r1[:], Lcmf[:])
        nc.vector.tensor_copy(Lcl[:], r1[:])

        # ---- helper: emit one accumulation group ----
        def mm_group(dest, contribs):
            n = len(contribs)
            for i, (lhsT, rhs) in enumerate(contribs):
                nc.tensor.matmul(dest, lhsT, rhs,
                                 start=(i == 0), stop=(i == n - 1))

        def gru_gates(grz, gihn, lc_in, bhn, h, tag):
            """gates + state update for one layer; h updated in place.
            grz psum [P,1024] already holds bias + gi_rz + gh_rz (bias was
            seeded by an identity-matmul), so sigmoid reads PSUM directly.
            gihn psum: [gi_n | gh_n]; lc_in/bhn are fp32 sbuf adds."""
            rr = wk.tile([P, H], f32, tag="rr", name=f"rr{tag}")
            nc.scalar.activation(rr[:], grz[:, 0:512], AF.Sigmoid)
            # off-critical-path adds overlap the sigmoid
            hn = wk.tile([P, H], f32, tag="hn", name=f"hn{tag}")
            nc.vector.tensor_add(hn[:], gihn[:, 512:1024], bhn)
            inn = wk.tile([P, H], f32, tag="inn", name=f"inn{tag}")
            nc.vector.tensor_add(inn[:], gihn[:, 0:512], lc_in)
            zz = wk.tile([P, H], f32, tag="zz", name=f"zz{tag}")
            nc.scalar.activation(zz[:], grz[:, 512:1024], AF.Sigmoid)
            rhn = wk.tile([P, H], f32, tag="rhn", name=f"rhn{tag}")
            nc.vector.tensor_mul(rhn[:], rr[:], hn[:])
            npre = wk.tile([P, H], f32, tag="npre", name=f"npre{tag}")
            nc.vector.tensor_add(npre[:], inn[:], rhn[:])
            nn = wk.tile([P, H], f32, tag="nn", name=f"nn{tag}")
            nc.scalar.activation(nn[:], npre[:], AF.Tanh)
            # h' = n + z*(h - n)
            dd = wk.tile([P, H], f32, tag="dd", name=f"dd{tag}")
            nc.vector.tensor_sub(dd[:], h[:], nn[:])
            zd = wk.tile([P, H], f32, tag="zd", name=f"zd{tag}")
            nc.vector.tensor_mul(zd[:], zz[:], dd[:])
            # final add per 128-chunk so each transpose starts asap
            for kc in range(4):
                sl = slice(kc * P, (kc + 1) * P)
                nc.vector.tensor_add(h[:, sl], nn[:, sl], zd[:, sl])

        def bias_seed(dest, parts, stop=False):
            """Seed a psum region with a replicated bias via identity
            matmuls. `parts` is an EXACT 3-way bf16 decomposition, so this
            is bit-identical to adding the fp32 bias; it starts the
            region's accumulation group as prefetchable PE work, removing
            a DVE add from the critical path."""
            n = dest.shape[-1]
            for ci in range(0, n, 512):
                w = min(512, n - ci)
                for pi, part in enumerate(parts):
                    nc.tensor.matmul(dest[:, ci:ci + w], identb[:],
                                     part[:, ci:ci + w], start=(pi == 0),
                                     stop=(stop and pi == len(parts) - 1))

        # ---- the T decode steps, software-pipelined so the PE never idles:
        # step t's hh0/gh1 matmuls are emitted before step t-1's argmax /
        # onehot tail, so the PE chews on them while DVE finishes t-1.
        # (t=0 works uniformly because state/ohT start zeroed.) ----
        def argmax_tail(t, tail, lg):
            """argmax(lg, psum) -> one-hot (first max wins) -> DMA + ohT."""
            mx = wk.tile([P, 1], f32, tag="mx", name=f"mx_{t}")
            nc.vector.reduce_max(mx[:], lg, axis=mybir.AxisListType.X)
            ohraw = wk.tile([P, V], f32, tag="ohraw", name=f"ohraw_{t}")
            nc.vector.tensor_scalar(ohraw[:], lg, mx[:, 0:1], None,
                                    op0=OP.is_equal)
            cum = wk.tile([P, V], f32, tag="cum", name=f"cum_{t}")
            nc.vector.tensor_tensor_scan(cum[:], ohraw[:], ohraw[:], 0.0,
                                         op0=OP.add, op1=OP.bypass)
            oh = wk.tile([P, V], f32, tag="oh", name=f"oh_{t}")
            nc.vector.scalar_tensor_tensor(oh[:], cum[:], 1.0, ohraw[:],
                                           op0=OP.is_equal, op1=OP.mult)
            nc.sync.dma_start(out_d[:, t, :], oh[:])
            if tail is not None:
                for v in range(2):
                    nc.tensor.transpose(tail[:, 256 + v * P:256 + (v + 1) * P],
                                        oh[:, ts(v, P)], ident[:])
                nc.scalar.copy(ohT[:, :, :].rearrange("p a b -> p (a b)"),
                               tail[:, 256:512])

        def l0_prefetch(t, g0rz, g0ihn):
            """Seeds + hh0 for step t (h-dependent parts skipped at t=0)."""
            bias_seed(g0rz, (Lch, Lcm, Lcl), stop=(t == 0))
            if t > 0:
                for j in range(2):   # hh0 rz (emb finishes the group later)
                    for kc in range(4):
                        nc.tensor.matmul(g0rz[:, ts(j, 512)], h0T[:, kc, :],
                                         whh0T[:, kc, ts(j, 512)],
                                         start=False, stop=False)
                mm_group(g0ihn[:, 512:1024],
                         [(h0T[:, kc, :], whh0T[:, kc, 1024:1536])
                          for kc in range(4)])
            else:
                nc.tensor.matmul(g0ihn[:, 512:1024], identb[:], zer[:],
                                 start=True, stop=True)
                nc.tensor.matmul(g0ihn[:, 0:512], identb[:], zer[:],
                                 start=True, stop=True)

        def l1_prefetch(t, g1rz):
            bias_seed(g1rz, (b1rzh, b1rzm, b1rzl))
            if t > 0:
                for j in range(2):   # gh1 rz (gi1 finishes the group later)
                    for kc in range(4):
                        nc.tensor.matmul(g1rz[:, ts(j, 512)], h1T[:, kc, :],
                                         whh1T[:, kc, ts(j, 512)],
                                         start=False, stop=False)

        # step-0 prefetch; later steps prefetch from inside iteration t-1
        g0rz = ps.tile([P, 1024], f32, tag="rz", name="g0rz_0")
        g0ihn = ps.tile([P, 1024], f32, tag="ihn", name="g0ihn_0")
        l0_prefetch(0, g0rz, g0ihn)
        g1rz = ps.tile([P, 1024], f32, tag="rz", name="g1rz_0")
        l1_prefetch(0, g1rz)

        prev_ihn = None
        prev_lg = None
        for t in range(T):
            # -- step t-1 tail: argmax -> one-hot -> DMA -> ohT --
            if t > 0:
                argmax_tail(t - 1, prev_ihn, prev_lg)

            # -- gh1 h_n (the t-1 tail above released this ihn slot) --
            g1ihn = ps.tile([P, 1024], f32, tag="ihn", name=f"g1ihn_{t}")
            if t > 0:
                mm_group(g1ihn[:, 512:1024],
                         [(h1T[:, kc, :], whh1T[:, kc, 1024:1536])
                          for kc in range(4)])
            else:
                nc.tensor.matmul(g1ihn[:, 512:1024], identb[:], zer[:],
                                 start=True, stop=True)

            # -- emb finishes layer0 groups (needs ohT from t-1 tail);
            #    EXACT 3-way bf16; regions complete in chain-priority
            #    order r -> i_n -> z (z is only needed at the blend) --
            if t > 0:
                for j in (0, None, 1):
                    if j is None:
                        mm_group(g0ihn[:, 0:512],
                                 [(ohT[:, v, :], hl[:, v, 1024:1536])
                                  for hl in (wembTh, wembTm, wembTl)
                                  for v in range(2)])
                        continue
                    for hl_i, hl in enumerate((wembTh, wembTm, wembTl)):
                        for vi, v in enumerate(range(2)):
                            nc.tensor.matmul(g0rz[:, ts(j, 512)], ohT[:, v, :],
                                             hl[:, v, ts(j, 512)],
                                             start=False,
                                             stop=(hl_i == 2 and vi == 1))

            # -- layer0 gates -> h0 (in place) --
            gru_gates(g0rz, g0ihn, Lc[:, 1024:1536], b0hn[:], h0, f"0_{t}")

            # -- h0'^T -> recycled g0ihn bank0 -> h0T (per-chunk) --
            for kc in range(4):
                nc.tensor.transpose(g0ihn[:, ts(kc, P)], h0[:, ts(kc, P)],
                                    ident[:])
                nc.scalar.copy(h0T[:, kc, :], g0ihn[:, ts(kc, P)])

            # -- gi1 (= h0' @ Wih1T), regions r -> i_n -> z --
            for j in (0, None, 1):
                if j is None:
                    mm_group(g1ihn[:, 0:512],
                             [(h0T[:, kc, :], wih1T[:, kc, 1024:1536])
                              for kc in range(4)])
                    continue
                for kc in range(4):
                    nc.tensor.matmul(g1rz[:, ts(j, 512)], h0T[:, kc, :],
                                     wih1T[:, kc, ts(j, 512)],
                                     start=False, stop=(kc == 3))

            # -- prefetch step t+1 layer0 (fills PE during l1 gates) --
            if t + 1 < T:
                ng0rz = ps.tile([P, 1024], f32, tag="rz", name=f"g0rz_{t+1}")
                ng0ihn = ps.tile([P, 1024], f32, tag="ihn",
                                 name=f"g0ihn_{t+1}")
                l0_prefetch(t + 1, ng0rz, ng0ihn)

            # -- layer1 gates -> h1 (in place) --
            gru_gates(g1rz, g1ihn, b1in[:], b1hn[:], h1, f"1_{t}")

            # -- h1'^T -> recycled g1ihn bank0 -> h1T --
            for kc in range(4):
                nc.tensor.transpose(g1ihn[:, ts(kc, P)], h1[:, ts(kc, P)],
                                    ident[:])
                nc.scalar.copy(h1T[:, kc, :], g1ihn[:, ts(kc, P)])

            # -- fc logits (+bias seed) -> recycled g1ihn cols [0:256] --
            bias_seed(g1ihn[:, 0:256], (bfch, bfcm, bfcl))
            for kc in range(4):
                nc.tensor.matmul(g1ihn[:, 0:256], h1T[:, kc, :],
                                 wfcT[:, kc, :], start=False, stop=(kc == 3))

            # -- prefetch step t+1 layer1 rz (needs the new h1T) --
            if t + 1 < T:
                ng1rz = ps.tile([P, 1024], f32, tag="rz", name=f"g1rz_{t+1}")
                l1_prefetch(t + 1, ng1rz)
                g0rz, g0ihn, g1rz = ng0rz, ng0ihn, ng1rz

            prev_ihn, prev_lg = g1ihn, g1ihn[:, 0:256]

        argmax_tail(T - 1, None, prev_lg)

    nc.compile()
    return nc


def prep_host_inputs(latent_vec, w_ih0, w_hh0, b_ih0, b_hh0,
                     w_ih_r, w_hh_r, b_ih_r, b_hh_r, w_fc, b_fc,
                     use_f32r=False):
    """Pure-layout host prep: transposes/reshapes + bias merge/replicate.
    Returns per-core in_maps."""
    import ml_dtypes
    f4 = np.float32
    bf = ml_dtypes.bfloat16

    def rep(v):  # replicate a [N] vector across the 128 partitions
        return np.ascontiguousarray(np.broadcast_to(v.astype(f4), (P, v.shape[0])))

    def split_bf16(a):  # EXACT 3-way bf16 split: a == h + m + l in fp32
        a = a.astype(f4)
        hi = a.astype(bf)
        r = a - hi.astype(f4)
        mid = r.astype(bf)
        lo = (r - mid.astype(f4)).astype(bf)
        return (np.ascontiguousarray(hi), np.ascontiguousarray(mid),
                np.ascontiguousarray(lo))

    wlatT = np.ascontiguousarray(w_ih0[:, :LAT].T.astype(f4)).reshape(2, P, G3)
    wembT = np.ascontiguousarray(w_ih0[:, LAT:].T.astype(f4)).reshape(2, P, G3)
    wembTh, wembTm, wembTl = split_bf16(wembT)
    whh0T = np.ascontiguousarray(w_hh0.T.astype(f4)).reshape(4, P, G3)
    wih1T = np.ascontiguousarray(w_ih_r[0].T.astype(f4)).reshape(4, P, G3)
    whh1T = np.ascontiguousarray(w_hh_r[0].T.astype(f4)).reshape(4, P, G3)
    wfcT = np.ascontiguousarray(w_fc.T.astype(f4)).reshape(4, P, V)

    blc_v = b_ih0.astype(f4).copy()
    blc_v[:1024] += b_hh0[:1024].astype(f4)
    b1rzh, b1rzm, b1rzl = split_bf16(rep((b_ih_r[0] + b_hh_r[0])[:1024]))
    bfch, bfcm, bfcl = split_bf16(rep(b_fc))
    common = dict(
        wlatT=wlatT, wembTh=wembTh, wembTm=wembTm, wembTl=wembTl,
        whh0T=whh0T, wih1T=wih1T, whh1T=whh1T,
        wfcT=wfcT, blc=rep(blc_v), b0hn=rep(b_hh0[1024:]),
        b1rzh=b1rzh, b1rzm=b1rzm, b1rzl=b1rzl,
        b1in=rep(b_ih_r[0][1024:]), b1hn=rep(b_hh_r[0][1024:]),
        bfch=bfch, bfcm=bfcm, bfcl=bfcl,
    )
    in_maps = []
    for c in range(N_CORES):
        m = dict(common)
        m["lat"] = np.ascontiguousarray(latent_vec[c * P:(c + 1) * P].astype(f4))
        in_maps.append(m)
    return in_maps


def kernel(**inputs):
    from concourse import bass_utils

    use_f32r = _CACHE.get("use_f32r", False)
    key = ("prog", T_FULL, use_f32r)
    if key not in _CACHE:
        _CACHE[key] = build_program(T_FULL, use_f32r=use_f32r)
    nc = _CACHE[key]

    in_maps = prep_host_inputs(
        np.asarray(inputs["latent_vec"]), np.asarray(inputs["w_ih0"]),
        np.asarray(inputs["w_hh0"]), np.asarray(inputs["b_ih0"]),
        np.asarray(inputs["b_hh0"]), np.asarray(inputs["w_ih_r"]),
        np.asarray(inputs["w_hh_r"]), np.asarray(inputs["b_ih_r"]),
        np.asarray(inputs["b_hh_r"]), np.asarray(inputs["w_fc"]),
        np.asarray(inputs["b_fc"]), use_f32r=use_f32r)

    res = bass_utils.run_bass_kernel_spmd(nc, in_maps, list(range(N_CORES)))
    out = np.concatenate([res.results[c]["out"] for c in range(N_CORES)], axis=0)
    return out.astype(np.float32)
